# revision 1
# baseline (speedup 1.0000x reference)
"""Multi-head attention (B=2, L=2048, D=1024, H=16, RoPE, softmax, out-proj)
on 8 Trainium2 NeuronCores.

Sharding: 2-way data parallel on batch x 4-way tensor parallel on heads.
Core c handles batch c//4 and heads 4*(c%4) .. 4*(c%4)+3. Each core:
  - projects its batch's activations with its head-slice of W_qkv
  - applies RoPE, computes S^T = K_rope Q_rope^T per head (transposed layout
    so softmax-normalizer and P.T@V need no on-chip transposes of P)
  - exp (no max subtraction: logits ~ N(0,1), |S| < ~6, exp is safe in fp32)
  - o~[q, 65] = P^T.T @ [V | 1]  (col 64 accumulates the softmax denominator)
  - normalize, transpose o, apply W_out slice -> partial output^T [1024, 2048]
  - ReduceScatter(add) over the 4 cores of the batch group -> [256, 2048]
Host reassembles the full [2, 2048, 1024] output.

All matmuls bf16 with fp32 PSUM accumulation; softmax in fp32 (PSUM) with
bf16 P storage.
"""

import numpy as np
import ml_dtypes
from contextlib import ExitStack

import concourse.bass as bass
import concourse.tile as tile
from concourse import bacc, mybir
from concourse.bass_utils import run_bass_kernel_spmd
from concourse.masks import make_identity

BF16 = mybir.dt.bfloat16
F32 = mybir.dt.float32

B, L, D = 2, 2048, 1024
H_TOT, H = 16, 4          # total heads, heads per core
HD, HF = 64, 32           # head dim, rope freqs
DL = H * HD               # local head dims per core = 256
P = 128                   # partitions
KT = L // P               # 16 k-tiles
QT_N = L // P             # 16 q-tiles
DK = D // P               # 8 contraction tiles over model dim
NCHUNK = 512              # moving-operand chunk
GRP = 4                   # k-tiles per PV accumulation group
ROPE_BASE = 10000.0
GROUPS = [[0, 1, 2, 3], [4, 5, 6, 7]]

_CACHED_NC = None


def _build_program():
    nc = bacc.Bacc("TRN2", target_bir_lowering=False, debug=False, num_devices=8)

    xT_ext = nc.dram_tensor("xT", [D, L], BF16, kind="ExternalInput")
    wqk_ext = nc.dram_tensor("wqkT", [D, 4 * P], BF16, kind="ExternalInput")
    wv_ext = nc.dram_tensor("wvT", [D, DL], BF16, kind="ExternalInput")
    wo_ext = nc.dram_tensor("woT", [DL, D], BF16, kind="ExternalInput")
    cos_ext = nc.dram_tensor("cosF", [P, L], F32, kind="ExternalInput")
    sin_ext = nc.dram_tensor("sinF", [P, L], F32, kind="ExternalInput")
    out_ext = nc.dram_tensor("out", [DL, L], F32, kind="ExternalOutput")

    partialT = nc.dram_tensor("partialT", [D, L], F32)
    scatT = nc.dram_tensor("scatT", [DL, L], F32)

    with tile.TileContext(nc) as tc:
        with ExitStack() as ctx:
            pers = ctx.enter_context(tc.tile_pool(name="pers", bufs=1))

            wqk = pers.tile([P, DK, 4 * P], BF16, tag="wqk")
            wv = pers.tile([P, DK, DL], BF16, tag="wv")
            wo = pers.tile([P, 2, D], BF16, tag="wo")
            qt = pers.tile([P, 2, L], BF16, tag="qt")     # head-contig Q^T
            kt_s = pers.tile([P, 2, L], BF16, tag="kt")   # head-contig K^T
            v1 = pers.tile([P, KT, H * (HD + 1)], BF16, tag="v1")  # [V | 1]
            o_acc = pers.tile([P, QT_N, H * (HD + 1)], F32, tag="oacc")
            o_nrm = pers.tile([P, QT_N, DL], BF16, tag="onrm")
            onT = pers.tile([P, 2, L], BF16, tag="onT")
            ident = pers.tile([P, P], BF16, tag="ident")

            for dk in range(DK):
                nc.sync.dma_start(out=wqk[:, dk, :], in_=wqk_ext[dk * P:(dk + 1) * P, :])
                nc.sync.dma_start(out=wv[:, dk, :], in_=wv_ext[dk * P:(dk + 1) * P, :])
            for t in range(2):
                nc.sync.dma_start(out=wo[:, t, :], in_=wo_ext[t * P:(t + 1) * P, :])
            make_identity(nc, ident[:])

            # ---------------- projections + rope ----------------
            with ExitStack() as pctx:
                pj = pctx.enter_context(tc.tile_pool(name="proj", bufs=1))
                tmp = pctx.enter_context(tc.tile_pool(name="ptmp", bufs=4))
                pp = pctx.enter_context(tc.tile_pool(name="pjps", bufs=4, space="PSUM"))

                xt = pj.tile([P, DK, L], BF16, tag="xt")
                cosf = pj.tile([P, L], F32, tag="cosf")
                sinf = pj.tile([P, L], F32, tag="sinf")
                qkr = pj.tile([P, 4, L], BF16, tag="qkr")  # qr1 qr2 kr1 kr2

                for dk in range(DK):
                    nc.sync.dma_start(out=xt[:, dk, :], in_=xT_ext[dk * P:(dk + 1) * P, :])
                nc.sync.dma_start(out=cosf[:], in_=cos_ext[:])
                nc.sync.dma_start(out=sinf[:], in_=sin_ext[:])

                # QK projection, x-chunks of 512; m: 0=qx1 1=qx2 2=kx1 3=kx2
                for c in range(L // NCHUNK):
                    xs = slice(c * NCHUNK, (c + 1) * NCHUNK)
                    ps_m = {}
                    for m in range(4):
                        pq = pp.tile([P, NCHUNK], F32, tag="qk")
                        for dk in range(DK):
                            nc.tensor.matmul(
                                pq[:], wqk[:, dk, m * P:(m + 1) * P], xt[:, dk, xs],
                                start=(dk == 0), stop=(dk == DK - 1))
                        ps_m[m] = pq
                        if m % 2 == 1:
                            # rope for this q/k pair of blocks
                            base = m - 1  # 0 for q, 2 for k
                            x1, x2 = ps_m[base], ps_m[base + 1]
                            t1 = tmp.tile([P, NCHUNK], F32, tag="t1")
                            t2 = tmp.tile([P, NCHUNK], F32, tag="t2")
                            nc.vector.tensor_mul(t1[:], x1[:], cosf[:, xs])
                            nc.vector.tensor_mul(t2[:], x2[:], sinf[:, xs])
                            nc.vector.tensor_sub(qkr[:, base, xs], t1[:], t2[:])
                            t3 = tmp.tile([P, NCHUNK], F32, tag="t1")
                            t4 = tmp.tile([P, NCHUNK], F32, tag="t2")
                            nc.vector.tensor_mul(t3[:], x1[:], sinf[:, xs])
                            nc.vector.tensor_mul(t4[:], x2[:], cosf[:, xs])
                            nc.vector.tensor_add(qkr[:, base + 1, xs], t3[:], t4[:])

                # V projection into [k, d] layout with ones column
                for k in range(KT):
                    pv = pp.tile([P, DL], F32, tag="v")
                    for dk in range(DK):
                        nc.tensor.matmul(
                            pv[:], xt[:, dk, k * P:(k + 1) * P], wv[:, dk, :],
                            start=(dk == 0), stop=(dk == DK - 1))
                    src3 = pv[:].rearrange("p (h d) -> p h d", h=H)
                    dst3 = v1[:, k, :].rearrange("p (h d) -> p h d", h=H)
                    nc.vector.tensor_copy(dst3[:, :, 0:HD], src3)
                    nc.vector.memset(dst3[:, :, HD:HD + 1], 1.0)

                # rearrange rope outputs to head-contiguous Q^T / K^T
                for h in range(H):
                    t, pb = h // 2, 64 * (h % 2)
                    hs = slice(32 * h, 32 * h + 32)
                    nc.sync.dma_start(out=qt[pb:pb + 32, t, :], in_=qkr[hs, 0, :])
                    nc.sync.dma_start(out=qt[pb + 32:pb + 64, t, :], in_=qkr[hs, 1, :])
                    nc.sync.dma_start(out=kt_s[pb:pb + 32, t, :], in_=qkr[hs, 2, :])
                    nc.sync.dma_start(out=kt_s[pb + 32:pb + 64, t, :], in_=qkr[hs, 3, :])

            # ---------------- attention ----------------
            with ExitStack() as actx:
                ptp = actx.enter_context(tc.tile_pool(name="ptp", bufs=20))
                stp = actx.enter_context(tc.tile_pool(name="stp", bufs=2, space="PSUM"))
                pvp = actx.enter_context(tc.tile_pool(name="pvp", bufs=4, space="PSUM"))

                for g in range(KT // GRP):
                    kts = range(g * GRP, (g + 1) * GRP)
                    pt_tiles = {}
                    for k in kts:
                        ks = slice(k * P, (k + 1) * P)
                        for t in range(2):  # head pairs (2t, 2t+1)
                            pt_a = ptp.tile([P, L], BF16, tag="pt")
                            pt_b = ptp.tile([P, L], BF16, tag="pt")
                            pt_tiles[(k, 2 * t)] = pt_a
                            pt_tiles[(k, 2 * t + 1)] = pt_b
                            for half in range(2):
                                st_a = stp.tile([P, L // 2], F32, tag="st")
                                st_b = stp.tile([P, L // 2], F32, tag="st")
                                for qc in range(2):
                                    qs = slice(half * (L // 2) + qc * NCHUNK,
                                               half * (L // 2) + (qc + 1) * NCHUNK)
                                    cs = slice(qc * NCHUNK, (qc + 1) * NCHUNK)
                                    nc.tensor.matmul(
                                        st_a[:, cs], kt_s[0:64, t, ks], qt[0:64, t, qs],
                                        start=True, stop=True, tile_position=(0, 0))
                                    nc.tensor.matmul(
                                        st_b[:, cs], kt_s[64:128, t, ks], qt[64:128, t, qs],
                                        start=True, stop=True, tile_position=(64, 0))
                                hs = slice(half * (L // 2), (half + 1) * (L // 2))
                                nc.scalar.activation(
                                    pt_a[:, hs], st_a[:], mybir.ActivationFunctionType.Exp)
                                nc.scalar.activation(
                                    pt_b[:, hs], st_b[:], mybir.ActivationFunctionType.Exp)

                    # PV for this group of k-tiles
                    for h in range(H):
                        vs = slice(h * (HD + 1), (h + 1) * (HD + 1))
                        for q in range(QT_N):
                            ob = pvp.tile([P, HD + 1], F32, tag="ob")
                            for j, k in enumerate(kts):
                                nc.tensor.matmul(
                                    ob[:], pt_tiles[(k, h)][:, q * P:(q + 1) * P],
                                    v1[:, k, vs],
                                    start=(j == 0), stop=(j == GRP - 1))
                            if g == 0:
                                nc.vector.tensor_copy(o_acc[:, q, vs], ob[:])
                            else:
                                nc.vector.tensor_add(o_acc[:, q, vs], o_acc[:, q, vs], ob[:])

            # ---------------- normalize + transpose + out-proj ----------------
            with ExitStack() as fctx:
                fin = fctx.enter_context(tc.tile_pool(name="fin", bufs=4))
                trp = fctx.enter_context(tc.tile_pool(name="trp", bufs=2, space="PSUM"))
                opp = fctx.enter_context(tc.tile_pool(name="opp", bufs=4, space="PSUM"))

                for q in range(QT_N):
                    oa3 = o_acc[:, q, :].rearrange("p (h c) -> p h c", h=H)
                    rec = fin.tile([P, H, 1], F32, tag="rec")
                    nc.vector.reciprocal(rec[:], oa3[:, :, HD:HD + 1])
                    for h in range(H):
                        nc.vector.tensor_scalar(
                            out=o_nrm[:, q, h * HD:(h + 1) * HD],
                            in0=oa3[:, h, 0:HD],
                            scalar1=rec[:, h, :], scalar2=None,
                            op0=mybir.AluOpType.mult)
                    for t in range(2):
                        ptr = trp.tile([P, P], BF16, tag="tr")
                        nc.tensor.transpose(ptr[:], o_nrm[:, q, t * P:(t + 1) * P], ident[:])
                        nc.vector.tensor_copy(onT[:, t, q * P:(q + 1) * P], ptr[:])

                for ot in range(DK):
                    for qc in range(L // NCHUNK):
                        qs = slice(qc * NCHUNK, (qc + 1) * NCHUNK)
                        po = opp.tile([P, NCHUNK], F32, tag="op")
                        for t in range(2):
                            nc.tensor.matmul(
                                po[:], wo[:, t, ot * P:(ot + 1) * P], onT[:, t, qs],
                                start=(t == 0), stop=(t == 1))
                        so = fin.tile([P, NCHUNK], F32, tag="so")
                        nc.vector.tensor_copy(so[:], po[:])
                        nc.sync.dma_start(out=partialT[ot * P:(ot + 1) * P, qs], in_=so[:])

                nc.gpsimd.collective_compute(
                    "ReduceScatter", mybir.AluOpType.add, replica_groups=GROUPS,
                    ins=[partialT[:]], outs=[scatT[:]])
                nc.sync.dma_start(out=out_ext[:], in_=scatT[:])

    nc.compile()
    return nc


def _prep_inputs(x, W_qkv, W_out):
    """Host-side sharding / layout prep -> per-core input maps."""
    Wq, Wk, Wv = W_qkv[0:D], W_qkv[D:2 * D], W_qkv[2 * D:3 * D]
    inv = 1.0 / (ROPE_BASE ** (np.arange(0, HD, 2, dtype=np.float64) / HD))
    pos = np.arange(L, dtype=np.float64)
    ang = pos[:, None] * inv[None, :]                     # [L, 32]
    cosF = np.tile(np.cos(ang).T, (H, 1)).astype(np.float32)  # [128, L]
    sinF = np.tile(np.sin(ang).T, (H, 1)).astype(np.float32)

    scale = float(HD) ** -0.5
    in_maps = []
    for c in range(8):
        b, g = c // 4, c % 4
        rows_x1 = np.array([64 * (4 * g + h) + 2 * f for h in range(H) for f in range(HF)])
        rows_x2 = rows_x1 + 1
        wqkT = np.concatenate([
            (scale * Wq[rows_x1]).T, (scale * Wq[rows_x2]).T,
            Wk[rows_x1].T, Wk[rows_x2].T], axis=1)        # [1024, 512]
        wvT = Wv[DL * g:DL * (g + 1)].T                   # [1024, 256]
        woT = W_out[:, DL * g:DL * (g + 1)].T             # [256, 1024]
        in_maps.append({
            "xT": np.ascontiguousarray(x[b].T).astype(ml_dtypes.bfloat16),
            "wqkT": np.ascontiguousarray(wqkT).astype(ml_dtypes.bfloat16),
            "wvT": np.ascontiguousarray(wvT).astype(ml_dtypes.bfloat16),
            "woT": np.ascontiguousarray(woT).astype(ml_dtypes.bfloat16),
            "cosF": cosF, "sinF": sinF,
        })
    return in_maps


def _run(in_maps, trace=False):
    global _CACHED_NC
    if _CACHED_NC is None:
        _CACHED_NC = _build_program()
    kw = dict(trace=True) if trace else {}
    return run_bass_kernel_spmd(_CACHED_NC, in_maps, list(range(8)), **kw)


def kernel(x, W_qkv, W_out, _trace=False):
    x = np.asarray(x, dtype=np.float32)
    W_qkv = np.asarray(W_qkv, dtype=np.float32)
    W_out = np.asarray(W_out, dtype=np.float32)
    res = _run(_prep_inputs(x, W_qkv, W_out), trace=_trace)
    out = np.empty((B, L, D), dtype=np.float32)
    for b in range(B):
        outT = np.concatenate([res.results[4 * b + j]["out"] for j in range(4)], axis=0)
        out[b] = outT.T
    if _trace:
        kernel.last_exec_time_ns = res.exec_time_ns
        kernel.last_trace = res.instructions_and_trace
    return out


# revision 2
# speedup vs baseline: 1.0407x; 1.0407x over previous
"""Multi-head attention (B=2, L=2048, D=1024, H=16, RoPE, softmax, out-proj)
on 8 Trainium2 NeuronCores.

Sharding: 2-way data parallel on batch x 4-way tensor parallel on heads.
Core c handles batch c//4 and heads 4*(c%4) .. 4*(c%4)+3. Each core:
  - projects its batch's activations with its head-slice of W_qkv
  - applies RoPE, computes S^T = K_rope Q_rope^T per head (transposed layout
    so the softmax normalizer and P.T@V need no on-chip transposes of P)
  - exp (no max subtraction: logits ~ N(0,1), |S| < ~6, exp is safe in fp32)
  - o~[q, 65] = P^T.T @ [V | 1] accumulated over all k in PSUM
    (col 64 = softmax denominator); normalized straight out of PSUM
  - transpose o, apply W_out slice -> partial output^T [1024, 2048]
  - ReduceScatter(add) over the batch group, chunked 4x along the sequence
    so the collective overlaps the tail of compute
Host reassembles the full [2, 2048, 1024] output.

All matmuls bf16 with fp32 PSUM accumulation; softmax in fp32 (PSUM) with
bf16 P storage.
"""

import numpy as np
import ml_dtypes
from contextlib import ExitStack

import concourse.bass as bass
import concourse.tile as tile
from concourse import bacc, mybir
from concourse.bass_utils import run_bass_kernel_spmd
from concourse.masks import make_identity

BF16 = mybir.dt.bfloat16
F32 = mybir.dt.float32

B, L, D = 2, 2048, 1024
H_TOT, H = 16, 4          # total heads, heads per core
HD, HF = 64, 32           # head dim, rope freqs
DL = H * HD               # local head dims per core = 256
P = 128                   # partitions
KT = L // P               # 16 k-tiles
QT_N = L // P             # 16 q-tiles
DK = D // P               # 8 contraction tiles over model dim
NCHUNK = 512              # moving-operand chunk
NQC = L // NCHUNK         # 4 sequence chunks
ROPE_BASE = 10000.0
GROUPS = [[0, 1, 2, 3], [4, 5, 6, 7]]

_CACHED_NC = None


def _build_program():
    nc = bacc.Bacc("TRN2", target_bir_lowering=False, debug=False, num_devices=8)

    xT_ext = nc.dram_tensor("xT", [D, L], BF16, kind="ExternalInput")
    wqk_ext = nc.dram_tensor("wqkT", [D, 4 * P], BF16, kind="ExternalInput")
    wv_ext = nc.dram_tensor("wvT", [D, DL], BF16, kind="ExternalInput")
    wo_ext = nc.dram_tensor("woT", [DL, D], BF16, kind="ExternalInput")
    cos_ext = nc.dram_tensor("cosF", [P, L], F32, kind="ExternalInput")
    sin_ext = nc.dram_tensor("sinF", [P, L], F32, kind="ExternalInput")
    out_ext = nc.dram_tensor("out", [DL, L], F32, kind="ExternalOutput")

    partials = [nc.dram_tensor(f"partialT{c}", [D, NCHUNK], F32) for c in range(NQC)]
    scats = [nc.dram_tensor(f"scatT{c}", [DL, NCHUNK], F32) for c in range(NQC)]

    with tile.TileContext(nc) as tc:
        with ExitStack() as ctx:
            pers = ctx.enter_context(tc.tile_pool(name="pers", bufs=1))

            wqk = pers.tile([P, DK, 4 * P], BF16, tag="wqk")
            wv = pers.tile([P, DK, DL], BF16, tag="wv")
            wo = pers.tile([P, 2, D], BF16, tag="wo")
            qt = pers.tile([P, 2, L], BF16, tag="qt")     # head-contig Q^T
            kt_s = pers.tile([P, 2, L], BF16, tag="kt")   # head-contig K^T
            v1 = pers.tile([P, KT, H * (HD + 1)], BF16, tag="v1")  # [V | 1]
            o_nrm = pers.tile([P, QT_N, DL], BF16, tag="onrm")
            onT = pers.tile([P, 2, L], BF16, tag="onT")
            ident = pers.tile([P, P], BF16, tag="ident")

            for dk in range(DK):
                nc.sync.dma_start(out=wqk[:, dk, :], in_=wqk_ext[dk * P:(dk + 1) * P, :])
                nc.sync.dma_start(out=wv[:, dk, :], in_=wv_ext[dk * P:(dk + 1) * P, :])
            for t in range(2):
                nc.sync.dma_start(out=wo[:, t, :], in_=wo_ext[t * P:(t + 1) * P, :])
            make_identity(nc, ident[:])

            # ---------------- projections + rope ----------------
            with ExitStack() as pctx:
                pj = pctx.enter_context(tc.tile_pool(name="proj", bufs=1))
                tmp = pctx.enter_context(tc.tile_pool(name="ptmp", bufs=4))
                pp = pctx.enter_context(tc.tile_pool(name="pjps", bufs=4, space="PSUM"))

                xt = pj.tile([P, DK, L], BF16, tag="xt")
                cosf = pj.tile([P, L], F32, tag="cosf")
                sinf = pj.tile([P, L], F32, tag="sinf")
                qkr = pj.tile([P, 4, L], BF16, tag="qkr")  # qr1 qr2 kr1 kr2

                for dk in range(DK):
                    nc.sync.dma_start(out=xt[:, dk, :], in_=xT_ext[dk * P:(dk + 1) * P, :])
                nc.sync.dma_start(out=cosf[:], in_=cos_ext[:])
                nc.sync.dma_start(out=sinf[:], in_=sin_ext[:])

                # QK projection, x-chunks of 512; m: 0=qx1 1=qx2 2=kx1 3=kx2
                for c in range(NQC):
                    xs = slice(c * NCHUNK, (c + 1) * NCHUNK)
                    ps_m = {}
                    for m in range(4):
                        pq = pp.tile([P, NCHUNK], F32, tag="qk")
                        for dk in range(DK):
                            nc.tensor.matmul(
                                pq[:], wqk[:, dk, m * P:(m + 1) * P], xt[:, dk, xs],
                                start=(dk == 0), stop=(dk == DK - 1))
                        ps_m[m] = pq
                        if m % 2 == 1:
                            base = m - 1  # 0 for q, 2 for k
                            x1, x2 = ps_m[base], ps_m[base + 1]
                            t1 = tmp.tile([P, NCHUNK], F32, tag="t1")
                            t2 = tmp.tile([P, NCHUNK], F32, tag="t2")
                            nc.vector.tensor_mul(t1[:], x1[:], cosf[:, xs])
                            nc.vector.tensor_mul(t2[:], x2[:], sinf[:, xs])
                            nc.vector.tensor_sub(qkr[:, base, xs], t1[:], t2[:])
                            t3 = tmp.tile([P, NCHUNK], F32, tag="t1")
                            t4 = tmp.tile([P, NCHUNK], F32, tag="t2")
                            nc.vector.tensor_mul(t3[:], x1[:], sinf[:, xs])
                            nc.vector.tensor_mul(t4[:], x2[:], cosf[:, xs])
                            nc.vector.tensor_add(qkr[:, base + 1, xs], t3[:], t4[:])

                # V projection into [k, d] layout with ones column
                for k in range(KT):
                    pv = pp.tile([P, DL], F32, tag="v")
                    for dk in range(DK):
                        nc.tensor.matmul(
                            pv[:], xt[:, dk, k * P:(k + 1) * P], wv[:, dk, :],
                            start=(dk == 0), stop=(dk == DK - 1))
                    src3 = pv[:].rearrange("p (h d) -> p h d", h=H)
                    dst3 = v1[:, k, :].rearrange("p (h d) -> p h d", h=H)
                    nc.vector.tensor_copy(dst3[:, :, 0:HD], src3)
                    nc.vector.memset(dst3[:, :, HD:HD + 1], 1.0)

                # rearrange rope outputs to head-contiguous Q^T / K^T
                for h in range(H):
                    t, pb = h // 2, 64 * (h % 2)
                    hs = slice(32 * h, 32 * h + 32)
                    nc.sync.dma_start(out=qt[pb:pb + 32, t, :], in_=qkr[hs, 0, :])
                    nc.sync.dma_start(out=qt[pb + 32:pb + 64, t, :], in_=qkr[hs, 1, :])
                    nc.sync.dma_start(out=kt_s[pb:pb + 32, t, :], in_=qkr[hs, 2, :])
                    nc.sync.dma_start(out=kt_s[pb + 32:pb + 64, t, :], in_=qkr[hs, 3, :])

            # ---------------- attention (head-outer) ----------------
            with ExitStack() as actx:
                ptp = actx.enter_context(tc.tile_pool(name="ptp", bufs=24))
                fin = actx.enter_context(tc.tile_pool(name="fin", bufs=8))
                stp = actx.enter_context(tc.tile_pool(name="stp", bufs=2, space="PSUM"))
                pvp = actx.enter_context(tc.tile_pool(name="pvp", bufs=4, space="PSUM"))

                for h in range(H):
                    t, pb = h // 2, 64 * (h % 2)
                    tpos = (pb, 0)
                    vs = slice(h * (HD + 1), (h + 1) * (HD + 1))
                    pts = []
                    for k in range(KT):
                        ks = slice(k * P, (k + 1) * P)
                        pt = ptp.tile([P, L], BF16, tag="pt")
                        pts.append(pt)
                        for half in range(2):
                            st = stp.tile([P, L // 2], F32, tag="st")
                            for qc in range(2):
                                qs = slice(half * (L // 2) + qc * NCHUNK,
                                           half * (L // 2) + (qc + 1) * NCHUNK)
                                cs = slice(qc * NCHUNK, (qc + 1) * NCHUNK)
                                nc.tensor.matmul(
                                    st[:, cs], kt_s[pb:pb + 64, t, ks],
                                    qt[pb:pb + 64, t, qs],
                                    start=True, stop=True, tile_position=tpos)
                            hs = slice(half * (L // 2), (half + 1) * (L // 2))
                            nc.scalar.activation(
                                pt[:, hs], st[:], mybir.ActivationFunctionType.Exp)

                    # PV: full-k accumulation chains, one per q-tile
                    for q in range(QT_N):
                        ob = pvp.tile([P, HD + 1], F32, tag="ob")
                        for k in range(KT):
                            nc.tensor.matmul(
                                ob[:], pts[k][:, q * P:(q + 1) * P], v1[:, k, vs],
                                start=(k == 0), stop=(k == KT - 1))
                        rec = fin.tile([P, 1], F32, tag="rec")
                        nc.vector.reciprocal(rec[:], ob[:, HD:HD + 1])
                        nc.vector.tensor_scalar(
                            out=o_nrm[:, q, h * HD:(h + 1) * HD],
                            in0=ob[:, 0:HD],
                            scalar1=rec[:], scalar2=None,
                            op0=mybir.AluOpType.mult)

            # ---------------- transpose + out-proj + collective ----------------
            with ExitStack() as fctx:
                fin2 = fctx.enter_context(tc.tile_pool(name="fin2", bufs=8))
                trp = fctx.enter_context(tc.tile_pool(name="trp", bufs=2, space="PSUM"))
                opp = fctx.enter_context(tc.tile_pool(name="opp", bufs=4, space="PSUM"))

                for q in range(QT_N):
                    for t in range(2):
                        ptr = trp.tile([P, P], BF16, tag="tr")
                        nc.tensor.transpose(ptr[:], o_nrm[:, q, t * P:(t + 1) * P], ident[:])
                        nc.vector.tensor_copy(onT[:, t, q * P:(q + 1) * P], ptr[:])

                for qc in range(NQC):
                    qs = slice(qc * NCHUNK, (qc + 1) * NCHUNK)
                    for ot in range(DK):
                        po = opp.tile([P, NCHUNK], F32, tag="op")
                        for t in range(2):
                            nc.tensor.matmul(
                                po[:], wo[:, t, ot * P:(ot + 1) * P], onT[:, t, qs],
                                start=(t == 0), stop=(t == 1))
                        so = fin2.tile([P, NCHUNK], F32, tag="so")
                        nc.vector.tensor_copy(so[:], po[:])
                        nc.sync.dma_start(out=partials[qc][ot * P:(ot + 1) * P, :], in_=so[:])
                    nc.gpsimd.collective_compute(
                        "ReduceScatter", mybir.AluOpType.add, replica_groups=GROUPS,
                        ins=[partials[qc][:]], outs=[scats[qc][:]])
                    nc.sync.dma_start(out=out_ext[:, qs], in_=scats[qc][:])

    nc.compile()
    return nc


def _prep_inputs(x, W_qkv, W_out):
    """Host-side sharding / layout prep -> per-core input maps."""
    Wq, Wk, Wv = W_qkv[0:D], W_qkv[D:2 * D], W_qkv[2 * D:3 * D]
    inv = 1.0 / (ROPE_BASE ** (np.arange(0, HD, 2, dtype=np.float64) / HD))
    pos = np.arange(L, dtype=np.float64)
    ang = pos[:, None] * inv[None, :]                     # [L, 32]
    cosF = np.tile(np.cos(ang).T, (H, 1)).astype(np.float32)  # [128, L]
    sinF = np.tile(np.sin(ang).T, (H, 1)).astype(np.float32)

    scale = float(HD) ** -0.5
    in_maps = []
    for c in range(8):
        b, g = c // 4, c % 4
        rows_x1 = np.array([64 * (4 * g + h) + 2 * f for h in range(H) for f in range(HF)])
        rows_x2 = rows_x1 + 1
        wqkT = np.concatenate([
            (scale * Wq[rows_x1]).T, (scale * Wq[rows_x2]).T,
            Wk[rows_x1].T, Wk[rows_x2].T], axis=1)        # [1024, 512]
        wvT = Wv[DL * g:DL * (g + 1)].T                   # [1024, 256]
        woT = W_out[:, DL * g:DL * (g + 1)].T             # [256, 1024]
        in_maps.append({
            "xT": np.ascontiguousarray(x[b].T).astype(ml_dtypes.bfloat16),
            "wqkT": np.ascontiguousarray(wqkT).astype(ml_dtypes.bfloat16),
            "wvT": np.ascontiguousarray(wvT).astype(ml_dtypes.bfloat16),
            "woT": np.ascontiguousarray(woT).astype(ml_dtypes.bfloat16),
            "cosF": cosF, "sinF": sinF,
        })
    return in_maps


def _run(in_maps, trace=False):
    global _CACHED_NC
    if _CACHED_NC is None:
        _CACHED_NC = _build_program()
    kw = dict(trace=True) if trace else {}
    return run_bass_kernel_spmd(_CACHED_NC, in_maps, list(range(8)), **kw)


def kernel(x, W_qkv, W_out, _trace=False):
    x = np.asarray(x, dtype=np.float32)
    W_qkv = np.asarray(W_qkv, dtype=np.float32)
    W_out = np.asarray(W_out, dtype=np.float32)
    res = _run(_prep_inputs(x, W_qkv, W_out), trace=_trace)
    out = np.empty((B, L, D), dtype=np.float32)
    for b in range(B):
        outT = np.concatenate([res.results[4 * b + j]["out"] for j in range(4)], axis=0)
        out[b] = outT.T
    if _trace:
        kernel.last_exec_time_ns = res.exec_time_ns
        kernel.last_trace = res.instructions_and_trace
    return out


# revision 5
# speedup vs baseline: 1.2382x; 1.1898x over previous
"""Multi-head attention (B=2, L=2048, D=1024, H=16, RoPE, softmax, out-proj)
on 8 Trainium2 NeuronCores.

Sharding: 2-way data parallel on batch x 4-way tensor parallel on heads.
Core c handles batch c//4 and heads 4*(c%4) .. 4*(c%4)+3. Each core:
  - projects its batch's activations with its head-slice of W_qkv
    (dk-outer accumulation so matmuls start as soon as x columns arrive)
  - applies RoPE, computes S^T = K_rope Q_rope^T per head in transposed
    layout (softmax normalizer and P.T@V then need no transposes of P);
    the K^T stationary operand is zero-padded to K=128 so the other head's
    lanes contribute exactly zero while the PE activity monitor sees a
    full-width matmul (K=64 matmuls leave the clock gated at half rate)
  - exp (no max subtraction: logits ~ N(0,1), |S| < ~6, exp is safe in fp32)
  - o~[q, 65] = P^T.T @ [V | 1] accumulated over all k in PSUM
    (col 64 = softmax denominator); normalized straight out of PSUM
  - q-half-outer loop: the first half's out-proj and ReduceScatter overlap
    the second half's attention
Host reassembles the full [2, 2048, 1024] output.

All matmuls bf16 with fp32 PSUM accumulation; softmax in fp32 (PSUM) with
bf16 P storage.
"""

import numpy as np
import ml_dtypes
from contextlib import ExitStack

import concourse.bass as bass
import concourse.tile as tile
from concourse import bacc, mybir
from concourse.bass_utils import run_bass_kernel_spmd
from concourse.masks import make_identity

BF16 = mybir.dt.bfloat16
F32 = mybir.dt.float32

B, L, D = 2, 2048, 1024
H_TOT, H = 16, 4          # total heads, heads per core
HD, HF = 64, 32           # head dim, rope freqs
DL = H * HD               # local head dims per core = 256
P = 128
KT = L // P               # 16 k-tiles
DK = D // P               # 8 contraction tiles over model dim
NCHUNK = 512
NQC = L // NCHUNK         # 4 sequence chunks (collective granularity)
QH = L // 2               # q half
ROPE_BASE = 10000.0
GROUPS = [[0, 1, 2, 3], [4, 5, 6, 7]]

_CACHED_NC = None


def _build_program():
    nc = bacc.Bacc("TRN2", target_bir_lowering=False, debug=False, num_devices=8)

    xT_ext = nc.dram_tensor("xT", [D, L], BF16, kind="ExternalInput")
    wqk_ext = nc.dram_tensor("wqkT", [D, 4 * P], BF16, kind="ExternalInput")
    wv_ext = nc.dram_tensor("wvT", [D, DL], BF16, kind="ExternalInput")
    wo_ext = nc.dram_tensor("woT", [DL, D], BF16, kind="ExternalInput")
    cos_ext = nc.dram_tensor("cosF", [P, L], F32, kind="ExternalInput")
    sin_ext = nc.dram_tensor("sinF", [P, L], F32, kind="ExternalInput")
    out_ext = nc.dram_tensor("out", [DL, L], F32, kind="ExternalOutput")

    partials = [nc.dram_tensor(f"partialT{c}", [D, NCHUNK], F32) for c in range(NQC)]
    scats = [nc.dram_tensor(f"scatT{c}", [DL, NCHUNK], F32) for c in range(NQC)]

    with tile.TileContext(nc) as tc:
        with ExitStack() as ctx:
            pers = ctx.enter_context(tc.tile_pool(name="pers", bufs=1))

            wv = pers.tile([P, DK, DL], BF16, tag="wv")
            wo = pers.tile([P, 2, D], BF16, tag="wo")
            qt = pers.tile([P, 2, L], BF16, tag="qt")      # head-contig Q^T
            ktz = pers.tile([P, H, L], BF16, tag="ktz")    # per-head zero-padded K^T
            v1 = pers.tile([P, KT, H * (HD + 1)], BF16, tag="v1")  # [V | 1]
            ident = pers.tile([P, P], BF16, tag="ident")

            nc.vector.memset(ktz[:], 0.0)
            for dk in range(DK):
                nc.sync.dma_start(out=wv[:, dk, :], in_=wv_ext[dk * P:(dk + 1) * P, :])
            for t in range(2):
                nc.sync.dma_start(out=wo[:, t, :], in_=wo_ext[t * P:(t + 1) * P, :])
            make_identity(nc, ident[:])

            xp = ctx.enter_context(tc.tile_pool(name="xp", bufs=1))
            xt = []  # [dk][cpair] -> [128, 1024]
            for dk in range(DK):
                row = []
                for cp in range(2):
                    x_t = xp.tile([P, 2 * NCHUNK], BF16, tag=f"xt{dk}_{cp}")
                    nc.sync.dma_start(
                        out=x_t[:],
                        in_=xT_ext[dk * P:(dk + 1) * P,
                                   cp * 2 * NCHUNK:(cp + 1) * 2 * NCHUNK])
                    row.append(x_t)
                xt.append(row)

            # ---------------- QK projection + rope ----------------
            with ExitStack() as pctx:
                pj = pctx.enter_context(tc.tile_pool(name="proj", bufs=1))
                tmp = pctx.enter_context(tc.tile_pool(name="ptmp", bufs=4))
                pp = pctx.enter_context(tc.tile_pool(name="pjps", bufs=1, space="PSUM"))

                wqk = pj.tile([P, DK, 4 * P], BF16, tag="wqk")
                cosf = pj.tile([P, L], F32, tag="cosf")
                sinf = pj.tile([P, L], F32, tag="sinf")
                qkr = pj.tile([P, 4, L], BF16, tag="qkr")  # qr1 qr2 kr1 kr2

                nc.sync.dma_start(out=cosf[:], in_=cos_ext[:])
                nc.sync.dma_start(out=sinf[:], in_=sin_ext[:])
                for dk in range(DK):
                    nc.sync.dma_start(out=wqk[:, dk, :],
                                      in_=wqk_ext[dk * P:(dk + 1) * P, :])

                # m: 0=qx1 1=qx2 2=kx1 3=kx2; dk-outer per chunk-pair
                for cp in range(2):
                    pq = {(c, m): pp.tile([P, NCHUNK], F32, tag=f"qk{c}{m}",
                                          name=f"pq_{cp}_{c}_{m}")
                          for c in range(2) for m in range(4)}
                    for dk in range(DK):
                        for c in range(2):
                            for m in range(4):
                                nc.tensor.matmul(
                                    pq[(c, m)][:],
                                    wqk[:, dk, m * P:(m + 1) * P],
                                    xt[dk][cp][:, c * NCHUNK:(c + 1) * NCHUNK],
                                    start=(dk == 0), stop=(dk == DK - 1))
                    for c in range(2):
                        xs = slice((2 * cp + c) * NCHUNK, (2 * cp + c + 1) * NCHUNK)
                        for base in (0, 2):
                            x1, x2 = pq[(c, base)], pq[(c, base + 1)]
                            t1 = tmp.tile([P, NCHUNK], F32, tag="t1")
                            t2 = tmp.tile([P, NCHUNK], F32, tag="t2")
                            nc.vector.tensor_mul(t1[:], x1[:], cosf[:, xs])
                            nc.vector.tensor_mul(t2[:], x2[:], sinf[:, xs])
                            nc.vector.tensor_sub(qkr[:, base, xs], t1[:], t2[:])
                            t3 = tmp.tile([P, NCHUNK], F32, tag="t1")
                            t4 = tmp.tile([P, NCHUNK], F32, tag="t2")
                            nc.vector.tensor_mul(t3[:], x1[:], sinf[:, xs])
                            nc.vector.tensor_mul(t4[:], x2[:], cosf[:, xs])
                            nc.vector.tensor_add(qkr[:, base + 1, xs], t3[:], t4[:])

                # head-contiguous Q^T (both heads per tile) and zero-padded K^T
                for h in range(H):
                    t, pb = h // 2, 64 * (h % 2)
                    hs = slice(32 * h, 32 * h + 32)
                    nc.sync.dma_start(out=qt[pb:pb + 32, t, :], in_=qkr[hs, 0, :])
                    nc.sync.dma_start(out=qt[pb + 32:pb + 64, t, :], in_=qkr[hs, 1, :])
                    nc.sync.dma_start(out=ktz[pb:pb + 32, h, :], in_=qkr[hs, 2, :])
                    nc.sync.dma_start(out=ktz[pb + 32:pb + 64, h, :], in_=qkr[hs, 3, :])

            # ---------------- attention + finish, q-half-outer ----------------
            with ExitStack() as actx:
                ptp = actx.enter_context(tc.tile_pool(name="ptp", bufs=1))
                fin = actx.enter_context(tc.tile_pool(name="fin", bufs=1))
                aps = actx.enter_context(tc.tile_pool(name="aps", bufs=1, space="PSUM"))

                first_v = True
                for qh in range(2):
                    qhs = slice(qh * QH, (qh + 1) * QH)
                    o_nrm = fin.tile([P, QH // P, DL], BF16, tag="onrm", bufs=2)
                    for h in range(H):
                        t = h // 2
                        vs = slice(h * (HD + 1), (h + 1) * (HD + 1))
                        pts = []
                        for k2 in range(KT // 2):
                            pt2 = ptp.tile([P, 2, QH], BF16, tag="pt", bufs=12)
                            pts.append(pt2)
                            for ki in range(2):
                                k = 2 * k2 + ki
                                ks = slice(k * P, (k + 1) * P)
                                st = aps.tile([P, QH], F32, tag="st", bufs=2)
                                for qc in range(2):
                                    cs = slice(qc * NCHUNK, (qc + 1) * NCHUNK)
                                    qs = slice(qh * QH + qc * NCHUNK,
                                               qh * QH + (qc + 1) * NCHUNK)
                                    nc.tensor.matmul(
                                        st[:, cs], ktz[:, h, ks], qt[:, t, qs],
                                        start=True, stop=True)
                                nc.scalar.activation(
                                    pt2[:, ki, :], st[:],
                                    mybir.ActivationFunctionType.Exp)

                        if first_v:
                            # V projection, emitted here so it fills the PE
                            # while the first head's exps run
                            first_v = False
                            for k in range(KT):
                                pv = aps.tile([P, DL], F32, tag="misc", bufs=2)
                                for dk in range(DK):
                                    nc.tensor.matmul(
                                        pv[:],
                                        xt[dk][k // 8][:, (k % 8) * P:(k % 8 + 1) * P],
                                        wv[:, dk, :],
                                        start=(dk == 0), stop=(dk == DK - 1))
                                src3 = pv[:].rearrange("p (h d) -> p h d", h=H)
                                dst3 = v1[:, k, :].rearrange("p (h d) -> p h d", h=H)
                                nc.vector.tensor_copy(dst3[:, :, 0:HD], src3)
                                nc.vector.memset(dst3[:, :, HD:HD + 1], 1.0)

                        # PV: full-k accumulation chains, one per q-tile
                        for q in range(QH // P):
                            ob = aps.tile([P, HD + 1], F32, tag="ob", bufs=2)
                            for k in range(KT):
                                nc.tensor.matmul(
                                    ob[:], pts[k // 2][:, k % 2, q * P:(q + 1) * P],
                                    v1[:, k, vs],
                                    start=(k == 0), stop=(k == KT - 1))
                            rec = fin.tile([P, 1], F32, tag="rec", bufs=4)
                            nc.vector.reciprocal(rec[:], ob[:, HD:HD + 1])
                            nc.vector.tensor_scalar(
                                out=o_nrm[:, q, h * HD:(h + 1) * HD],
                                in0=ob[:, 0:HD],
                                scalar1=rec[:], scalar2=None,
                                op0=mybir.AluOpType.mult)

                    # transpose + out-proj + collective for this half
                    onT = fin.tile([P, 2, QH], BF16, tag="onT", bufs=2)
                    for q in range(QH // P):
                        for t in range(2):
                            ptr = aps.tile([P, P], BF16, tag="misc", bufs=2)
                            nc.tensor.transpose(
                                ptr[:], o_nrm[:, q, t * P:(t + 1) * P], ident[:])
                            nc.vector.tensor_copy(onT[:, t, q * P:(q + 1) * P], ptr[:])

                    for qcw in range(2):
                        qc = qh * 2 + qcw
                        ws = slice(qcw * NCHUNK, (qcw + 1) * NCHUNK)
                        for ot in range(DK):
                            po = aps.tile([P, NCHUNK], F32, tag="misc", bufs=2)
                            for t in range(2):
                                nc.tensor.matmul(
                                    po[:], wo[:, t, ot * P:(ot + 1) * P], onT[:, t, ws],
                                    start=(t == 0), stop=(t == 1))
                            so = fin.tile([P, NCHUNK], F32, tag="so", bufs=4)
                            nc.vector.tensor_copy(so[:], po[:])
                            nc.sync.dma_start(
                                out=partials[qc][ot * P:(ot + 1) * P, :], in_=so[:])
                        nc.gpsimd.collective_compute(
                            "ReduceScatter", mybir.AluOpType.add,
                            replica_groups=GROUPS,
                            ins=[partials[qc][:]], outs=[scats[qc][:]])
                        nc.sync.dma_start(
                            out=out_ext[:, qc * NCHUNK:(qc + 1) * NCHUNK],
                            in_=scats[qc][:])

    nc.compile()
    return nc


def _prep_inputs(x, W_qkv, W_out):
    """Host-side sharding / layout prep -> per-core input maps."""
    Wq, Wk, Wv = W_qkv[0:D], W_qkv[D:2 * D], W_qkv[2 * D:3 * D]
    inv = 1.0 / (ROPE_BASE ** (np.arange(0, HD, 2, dtype=np.float64) / HD))
    pos = np.arange(L, dtype=np.float64)
    ang = pos[:, None] * inv[None, :]                     # [L, 32]
    cosF = np.tile(np.cos(ang).T, (H, 1)).astype(np.float32)  # [128, L]
    sinF = np.tile(np.sin(ang).T, (H, 1)).astype(np.float32)

    scale = float(HD) ** -0.5
    in_maps = []
    for c in range(8):
        b, g = c // 4, c % 4
        rows_x1 = np.array([64 * (4 * g + h) + 2 * f for h in range(H) for f in range(HF)])
        rows_x2 = rows_x1 + 1
        wqkT = np.concatenate([
            (scale * Wq[rows_x1]).T, (scale * Wq[rows_x2]).T,
            Wk[rows_x1].T, Wk[rows_x2].T], axis=1)        # [1024, 512]
        wvT = Wv[DL * g:DL * (g + 1)].T                   # [1024, 256]
        woT = W_out[:, DL * g:DL * (g + 1)].T             # [256, 1024]
        in_maps.append({
            "xT": np.ascontiguousarray(x[b].T).astype(ml_dtypes.bfloat16),
            "wqkT": np.ascontiguousarray(wqkT).astype(ml_dtypes.bfloat16),
            "wvT": np.ascontiguousarray(wvT).astype(ml_dtypes.bfloat16),
            "woT": np.ascontiguousarray(woT).astype(ml_dtypes.bfloat16),
            "cosF": cosF, "sinF": sinF,
        })
    return in_maps


def _run(in_maps, trace=False):
    global _CACHED_NC
    if _CACHED_NC is None:
        _CACHED_NC = _build_program()
    kw = dict(trace=True) if trace else {}
    return run_bass_kernel_spmd(_CACHED_NC, in_maps, list(range(8)), **kw)


def kernel(x, W_qkv, W_out, _trace=False):
    x = np.asarray(x, dtype=np.float32)
    W_qkv = np.asarray(W_qkv, dtype=np.float32)
    W_out = np.asarray(W_out, dtype=np.float32)
    res = _run(_prep_inputs(x, W_qkv, W_out), trace=_trace)
    out = np.empty((B, L, D), dtype=np.float32)
    for b in range(B):
        outT = np.concatenate([res.results[4 * b + j]["out"] for j in range(4)], axis=0)
        out[b] = outT.T
    if _trace:
        kernel.last_exec_time_ns = res.exec_time_ns
        kernel.last_trace = res.instructions_and_trace
    return out


# revision 6
# speedup vs baseline: 1.2507x; 1.0101x over previous
"""Multi-head attention (B=2, L=2048, D=1024, H=16, RoPE, softmax, out-proj)
on 8 Trainium2 NeuronCores.

Sharding: 2-way data parallel on batch x 4-way tensor parallel on heads.
Core c handles batch c//4 and heads 4*(c%4) .. 4*(c%4)+3. Each core:
  - projects its batch's activations with its head-slice of W_qkv
    (dk-outer accumulation so matmuls start as soon as x columns arrive)
  - applies RoPE, computes S^T = K_rope Q_rope^T per head in transposed
    layout (softmax normalizer and P.T@V then need no transposes of P);
    the K^T stationary operand is zero-padded to K=128 so the other head's
    lanes contribute exactly zero while the PE activity monitor sees a
    full-width matmul (K=64 matmuls leave the clock gated at half rate)
  - exp (no max subtraction: logits ~ N(0,1), |S| < ~6, exp is safe in fp32)
  - o~[q, 65] = P^T.T @ [V | 1] accumulated over all k in PSUM
    (col 64 = softmax denominator); normalized straight out of PSUM
  - q-half-outer loop: the first half's out-proj and ReduceScatter overlap
    the second half's attention
Host reassembles the full [2, 2048, 1024] output.

All matmuls bf16 with fp32 PSUM accumulation; softmax in fp32 (PSUM) with
bf16 P storage.
"""

import numpy as np
import ml_dtypes
from contextlib import ExitStack

import concourse.bass as bass
import concourse.tile as tile
from concourse import bacc, mybir
from concourse.bass_utils import run_bass_kernel_spmd
from concourse.masks import make_identity

BF16 = mybir.dt.bfloat16
F32 = mybir.dt.float32

B, L, D = 2, 2048, 1024
H_TOT, H = 16, 4          # total heads, heads per core
HD, HF = 64, 32           # head dim, rope freqs
DL = H * HD               # local head dims per core = 256
P = 128
KT = L // P               # 16 k-tiles
DK = D // P               # 8 contraction tiles over model dim
NCHUNK = 512
NQC = L // NCHUNK         # 4 sequence chunks (collective granularity)
QH = L // 2               # q half
ROPE_BASE = 10000.0
GROUPS = [[0, 1, 2, 3], [4, 5, 6, 7]]

_CACHED_NC = None


def _build_program():
    nc = bacc.Bacc("TRN2", target_bir_lowering=False, debug=False, num_devices=8)

    xT_ext = nc.dram_tensor("xT", [DK, 2, P, 2 * NCHUNK], BF16, kind="ExternalInput")
    wqk_ext = nc.dram_tensor("wqkT", [DK, P, 4 * P], BF16, kind="ExternalInput")
    wv_ext = nc.dram_tensor("wvT", [DK, P, DL], BF16, kind="ExternalInput")
    wo_ext = nc.dram_tensor("woT", [2, P, D], BF16, kind="ExternalInput")
    cos_ext = nc.dram_tensor("cosF", [P, L], F32, kind="ExternalInput")
    sin_ext = nc.dram_tensor("sinF", [P, L], F32, kind="ExternalInput")
    out_ext = nc.dram_tensor("out", [DL, L], F32, kind="ExternalOutput")

    partials = [nc.dram_tensor(f"partialT{c}", [D, QH], BF16) for c in range(2)]
    scats = [nc.dram_tensor(f"scatT{c}", [DL, QH], BF16) for c in range(2)]

    with tile.TileContext(nc) as tc:
        with ExitStack() as ctx:
            pers = ctx.enter_context(tc.tile_pool(name="pers", bufs=1))

            wv = pers.tile([P, DK, DL], BF16, tag="wv")
            wo = pers.tile([P, 2, D], BF16, tag="wo")
            qt = pers.tile([P, 2, L], BF16, tag="qt")      # head-contig Q^T
            ktz = pers.tile([P, H, L], BF16, tag="ktz")    # per-head zero-padded K^T
            v1 = pers.tile([P, KT, H * (HD + 1)], BF16, tag="v1")  # [V | 1]
            ident = pers.tile([P, P], BF16, tag="ident")

            xp = ctx.enter_context(tc.tile_pool(name="xp", bufs=1))
            xt = []  # [dk][cpair] -> [128, 1024]
            for dk in range(DK):
                row = []
                for cp in range(2):
                    x_t = xp.tile([P, 2 * NCHUNK], BF16, tag=f"xt{dk}_{cp}")
                    nc.sync.dma_start(out=x_t[:], in_=xT_ext[dk, cp])
                    row.append(x_t)
                xt.append(row)
            nc.vector.memset(ktz[:], 0.0)
            for dk in range(DK):
                nc.sync.dma_start(out=wv[:, dk, :], in_=wv_ext[dk])
            for t in range(2):
                nc.sync.dma_start(out=wo[:, t, :], in_=wo_ext[t])
            make_identity(nc, ident[:])

            # ---------------- QK projection + rope ----------------
            with ExitStack() as pctx:
                pj = pctx.enter_context(tc.tile_pool(name="proj", bufs=1))
                tmp = pctx.enter_context(tc.tile_pool(name="ptmp", bufs=4))
                pp = pctx.enter_context(tc.tile_pool(name="pjps", bufs=1, space="PSUM"))

                wqk = pj.tile([P, DK, 4 * P], BF16, tag="wqk")
                cosf = pj.tile([P, L], F32, tag="cosf")
                sinf = pj.tile([P, L], F32, tag="sinf")
                qkr = pj.tile([P, 4, L], BF16, tag="qkr")  # qr1 qr2 kr1 kr2

                nc.sync.dma_start(out=cosf[:], in_=cos_ext[:])
                nc.sync.dma_start(out=sinf[:], in_=sin_ext[:])
                for dk in range(DK):
                    nc.sync.dma_start(out=wqk[:, dk, :], in_=wqk_ext[dk])

                # m: 0=qx1 1=qx2 2=kx1 3=kx2; dk-outer per chunk-pair
                for cp in range(2):
                    pq = {(c, m): pp.tile([P, NCHUNK], F32, tag=f"qk{c}{m}",
                                          name=f"pq_{cp}_{c}_{m}")
                          for c in range(2) for m in range(4)}
                    for dk in range(DK):
                        for c in range(2):
                            for m in range(4):
                                nc.tensor.matmul(
                                    pq[(c, m)][:],
                                    wqk[:, dk, m * P:(m + 1) * P],
                                    xt[dk][cp][:, c * NCHUNK:(c + 1) * NCHUNK],
                                    start=(dk == 0), stop=(dk == DK - 1))
                    for c in range(2):
                        xs = slice((2 * cp + c) * NCHUNK, (2 * cp + c + 1) * NCHUNK)
                        for base in (0, 2):
                            x1, x2 = pq[(c, base)], pq[(c, base + 1)]
                            t1 = tmp.tile([P, NCHUNK], F32, tag="t1")
                            t2 = tmp.tile([P, NCHUNK], F32, tag="t2")
                            nc.vector.tensor_mul(t1[:], x1[:], cosf[:, xs])
                            nc.vector.tensor_mul(t2[:], x2[:], sinf[:, xs])
                            nc.vector.tensor_sub(qkr[:, base, xs], t1[:], t2[:])
                            t3 = tmp.tile([P, NCHUNK], F32, tag="t1")
                            t4 = tmp.tile([P, NCHUNK], F32, tag="t2")
                            nc.vector.tensor_mul(t3[:], x1[:], sinf[:, xs])
                            nc.vector.tensor_mul(t4[:], x2[:], cosf[:, xs])
                            nc.vector.tensor_add(qkr[:, base + 1, xs], t3[:], t4[:])

                # head-contiguous Q^T (both heads per tile) and zero-padded K^T
                for h in range(H):
                    t, pb = h // 2, 64 * (h % 2)
                    hs = slice(32 * h, 32 * h + 32)
                    nc.sync.dma_start(out=qt[pb:pb + 32, t, :], in_=qkr[hs, 0, :])
                    nc.sync.dma_start(out=qt[pb + 32:pb + 64, t, :], in_=qkr[hs, 1, :])
                    nc.sync.dma_start(out=ktz[pb:pb + 32, h, :], in_=qkr[hs, 2, :])
                    nc.sync.dma_start(out=ktz[pb + 32:pb + 64, h, :], in_=qkr[hs, 3, :])

            # ---------------- attention + finish, q-half-outer ----------------
            with ExitStack() as actx:
                ptp = actx.enter_context(tc.tile_pool(name="ptp", bufs=1))
                fin = actx.enter_context(tc.tile_pool(name="fin", bufs=1))
                aps = actx.enter_context(tc.tile_pool(name="aps", bufs=1, space="PSUM"))

                first_v = True
                for qh in range(2):
                    qhs = slice(qh * QH, (qh + 1) * QH)
                    o_nrm = fin.tile([P, QH // P, DL], BF16, tag="onrm", bufs=2)
                    for h in range(H):
                        t = h // 2
                        vs = slice(h * (HD + 1), (h + 1) * (HD + 1))
                        pts = []
                        for k2 in range(KT // 2):
                            pt2 = ptp.tile([P, 2, QH], BF16, tag="pt", bufs=12)
                            pts.append(pt2)
                            for ki in range(2):
                                k = 2 * k2 + ki
                                ks = slice(k * P, (k + 1) * P)
                                st = aps.tile([P, QH], F32, tag="st", bufs=2)
                                for qc in range(2):
                                    cs = slice(qc * NCHUNK, (qc + 1) * NCHUNK)
                                    qs = slice(qh * QH + qc * NCHUNK,
                                               qh * QH + (qc + 1) * NCHUNK)
                                    nc.tensor.matmul(
                                        st[:, cs], ktz[:, h, ks], qt[:, t, qs],
                                        start=True, stop=True)
                                nc.scalar.activation(
                                    pt2[:, ki, :], st[:],
                                    mybir.ActivationFunctionType.Exp)

                        if first_v:
                            # V projection, emitted here so it fills the PE
                            # while the first head's exps run
                            first_v = False
                            for k in range(KT):
                                pv = aps.tile([P, DL], F32, tag="misc", bufs=2)
                                for dk in range(DK):
                                    nc.tensor.matmul(
                                        pv[:],
                                        xt[dk][k // 8][:, (k % 8) * P:(k % 8 + 1) * P],
                                        wv[:, dk, :],
                                        start=(dk == 0), stop=(dk == DK - 1))
                                src3 = pv[:].rearrange("p (h d) -> p h d", h=H)
                                dst3 = v1[:, k, :].rearrange("p (h d) -> p h d", h=H)
                                nc.vector.tensor_copy(dst3[:, :, 0:HD], src3)
                                nc.vector.memset(dst3[:, :, HD:HD + 1], 1.0)

                        # PV: full-k accumulation chains, one per q-tile
                        for q in range(QH // P):
                            ob = aps.tile([P, HD + 1], F32, tag="ob", bufs=2)
                            for k in range(KT):
                                nc.tensor.matmul(
                                    ob[:], pts[k // 2][:, k % 2, q * P:(q + 1) * P],
                                    v1[:, k, vs],
                                    start=(k == 0), stop=(k == KT - 1))
                            rec = fin.tile([P, 1], F32, tag="rec", bufs=4)
                            nc.vector.reciprocal(rec[:], ob[:, HD:HD + 1])
                            nc.vector.tensor_scalar(
                                out=o_nrm[:, q, h * HD:(h + 1) * HD],
                                in0=ob[:, 0:HD],
                                scalar1=rec[:], scalar2=None,
                                op0=mybir.AluOpType.mult)

                    # transpose + out-proj + collective for this half
                    onT = fin.tile([P, 2, QH], BF16, tag="onT", bufs=2)
                    for q in range(QH // P):
                        for t in range(2):
                            ptr = aps.tile([P, P], BF16, tag="misc", bufs=2)
                            nc.tensor.transpose(
                                ptr[:], o_nrm[:, q, t * P:(t + 1) * P], ident[:])
                            nc.vector.tensor_copy(onT[:, t, q * P:(q + 1) * P], ptr[:])

                    for qcw in range(2):
                        ws = slice(qcw * NCHUNK, (qcw + 1) * NCHUNK)
                        for ot in range(DK):
                            po = aps.tile([P, NCHUNK], F32, tag="misc", bufs=2)
                            for t in range(2):
                                nc.tensor.matmul(
                                    po[:], wo[:, t, ot * P:(ot + 1) * P], onT[:, t, ws],
                                    start=(t == 0), stop=(t == 1))
                            so = fin.tile([P, NCHUNK], BF16, tag="so", bufs=4)
                            nc.vector.tensor_copy(so[:], po[:])
                            nc.sync.dma_start(
                                out=partials[qh][ot * P:(ot + 1) * P, ws], in_=so[:])
                        del po, so
                    nc.gpsimd.collective_compute(
                        "ReduceScatter", mybir.AluOpType.add,
                        replica_groups=GROUPS,
                        ins=[partials[qh][:]], outs=[scats[qh][:]])
                    sc = fin.tile([P, 2, QH], BF16, tag="sc", bufs=2)
                    scf = fin.tile([P, 2, QH], F32, tag="scf", bufs=2)
                    for t in range(2):
                        nc.sync.dma_start(out=sc[:, t, :],
                                          in_=scats[qh][t * P:(t + 1) * P, :])
                    nc.vector.tensor_copy(scf[:], sc[:])
                    for t in range(2):
                        nc.sync.dma_start(
                            out=out_ext[t * P:(t + 1) * P, qhs], in_=scf[:, t, :])

    nc.compile()
    return nc


def _prep_inputs(x, W_qkv, W_out):
    """Host-side sharding / layout prep -> per-core input maps."""
    Wq, Wk, Wv = W_qkv[0:D], W_qkv[D:2 * D], W_qkv[2 * D:3 * D]
    inv = 1.0 / (ROPE_BASE ** (np.arange(0, HD, 2, dtype=np.float64) / HD))
    pos = np.arange(L, dtype=np.float64)
    ang = pos[:, None] * inv[None, :]                     # [L, 32]
    cosF = np.tile(np.cos(ang).T, (H, 1)).astype(np.float32)  # [128, L]
    sinF = np.tile(np.sin(ang).T, (H, 1)).astype(np.float32)

    scale = float(HD) ** -0.5
    in_maps = []
    for c in range(8):
        b, g = c // 4, c % 4
        rows_x1 = np.array([64 * (4 * g + h) + 2 * f for h in range(H) for f in range(HF)])
        rows_x2 = rows_x1 + 1
        wqkT = np.concatenate([
            (scale * Wq[rows_x1]).T, (scale * Wq[rows_x2]).T,
            Wk[rows_x1].T, Wk[rows_x2].T], axis=1)        # [1024, 512]
        wvT = Wv[DL * g:DL * (g + 1)].T                   # [1024, 256]
        woT = W_out[:, DL * g:DL * (g + 1)].T             # [256, 1024]
        xTt = x[b].T.reshape(DK, P, 2, 2 * NCHUNK).transpose(0, 2, 1, 3)
        in_maps.append({
            "xT": np.ascontiguousarray(xTt).astype(ml_dtypes.bfloat16),
            "wqkT": np.ascontiguousarray(wqkT.reshape(DK, P, 4 * P)).astype(ml_dtypes.bfloat16),
            "wvT": np.ascontiguousarray(wvT.reshape(DK, P, DL)).astype(ml_dtypes.bfloat16),
            "woT": np.ascontiguousarray(woT.reshape(2, P, D)).astype(ml_dtypes.bfloat16),
            "cosF": cosF, "sinF": sinF,
        })
    return in_maps


def _run(in_maps, trace=False):
    global _CACHED_NC
    if _CACHED_NC is None:
        _CACHED_NC = _build_program()
    kw = dict(trace=True) if trace else {}
    return run_bass_kernel_spmd(_CACHED_NC, in_maps, list(range(8)), **kw)


def kernel(x, W_qkv, W_out, _trace=False):
    x = np.asarray(x, dtype=np.float32)
    W_qkv = np.asarray(W_qkv, dtype=np.float32)
    W_out = np.asarray(W_out, dtype=np.float32)
    res = _run(_prep_inputs(x, W_qkv, W_out), trace=_trace)
    out = np.empty((B, L, D), dtype=np.float32)
    for b in range(B):
        outT = np.concatenate([res.results[4 * b + j]["out"] for j in range(4)], axis=0)
        out[b] = outT.T
    if _trace:
        kernel.last_exec_time_ns = res.exec_time_ns
        kernel.last_trace = res.instructions_and_trace
    return out


# revision 7
# speedup vs baseline: 1.4037x; 1.1223x over previous
"""Multi-head attention (B=2, L=2048, D=1024, H=16, RoPE, softmax, out-proj)
on 8 Trainium2 NeuronCores.

Sharding: 2-way data parallel on batch x 4-way tensor parallel on heads.
Core c handles batch c//4 and heads 4*(c%4) .. 4*(c%4)+3. Each core:
  - projects its batch's activations with its head-slice of W_qkv
    (dk-outer accumulation so matmuls start as soon as x columns arrive)
  - applies RoPE, computes S^T = K_rope Q_rope^T per head in transposed
    layout (softmax normalizer and P.T@V then need no transposes of P);
    the K^T stationary operand is zero-padded to K=128 so the other head's
    lanes contribute exactly zero while the PE activity monitor sees a
    full-width matmul (K=64 matmuls leave the clock gated at half rate)
  - exp (no max subtraction: logits ~ N(0,1), |S| < ~6, exp is safe in fp32)
  - o~[q, 65] = P^T.T @ [V | 1] accumulated over all k in PSUM
    (col 64 = softmax denominator); normalized straight out of PSUM
  - q-half-outer loop: the first half's out-proj and ReduceScatter overlap
    the second half's attention
Host reassembles the full [2, 2048, 1024] output.

All matmuls bf16 with fp32 PSUM accumulation; softmax in fp32 (PSUM) with
bf16 P storage.
"""

import numpy as np
import ml_dtypes
from contextlib import ExitStack

import concourse.bass as bass
import concourse.tile as tile
from concourse import bacc, mybir
from concourse.bass_utils import run_bass_kernel_spmd
from concourse.masks import make_identity

BF16 = mybir.dt.bfloat16
F32 = mybir.dt.float32

B, L, D = 2, 2048, 1024
H_TOT, H = 16, 4          # total heads, heads per core
HD, HF = 64, 32           # head dim, rope freqs
DL = H * HD               # local head dims per core = 256
P = 128
KT = L // P               # 16 k-tiles
DK = D // P               # 8 contraction tiles over model dim
NCHUNK = 512
NQC = L // NCHUNK         # 4 sequence chunks (collective granularity)
QH = L // 2               # q half
ROPE_BASE = 10000.0
GROUPS = [[0, 1, 2, 3], [4, 5, 6, 7]]

_CACHED_NC = None


def _build_program():
    nc = bacc.Bacc("TRN2", target_bir_lowering=False, debug=False, num_devices=8)

    xT_ext = nc.dram_tensor("xT", [DK, 2, P, 2 * NCHUNK], BF16, kind="ExternalInput")
    wqk_ext = nc.dram_tensor("wqkT", [DK, P, 4 * P], BF16, kind="ExternalInput")
    wv_ext = nc.dram_tensor("wvT", [DK, P, DL], BF16, kind="ExternalInput")
    wo_ext = nc.dram_tensor("woT", [2, P, D], BF16, kind="ExternalInput")
    cos_ext = nc.dram_tensor("cosF", [P, L], F32, kind="ExternalInput")
    sin_ext = nc.dram_tensor("sinF", [P, L], F32, kind="ExternalInput")
    out_ext = nc.dram_tensor("out", [DL, L], F32, kind="ExternalOutput")

    partials = [nc.dram_tensor(f"partialT{c}", [D, QH], BF16) for c in range(2)]
    scats = [nc.dram_tensor(f"scatT{c}", [DL, QH], BF16) for c in range(2)]

    with tile.TileContext(nc) as tc:
        with ExitStack() as ctx:
            pers = ctx.enter_context(tc.tile_pool(name="pers", bufs=1))

            wv = pers.tile([P, DK, DL], BF16, tag="wv")
            wo = pers.tile([P, 2, D], BF16, tag="wo")
            qt = pers.tile([P, 2, L], BF16, tag="qt")      # head-contig Q^T
            ktz = pers.tile([P, H, L], BF16, tag="ktz")    # per-head zero-padded K^T
            v1 = pers.tile([P, KT, H * (HD + 1)], BF16, tag="v1")  # [V | 1]
            ident = pers.tile([P, P], BF16, tag="ident")

            xp = ctx.enter_context(tc.tile_pool(name="xp", bufs=1))
            xt = []  # [dk][cpair] -> [128, 1024]
            for dk in range(DK):
                row = []
                for cp in range(2):
                    x_t = xp.tile([P, 2 * NCHUNK], BF16, tag=f"xt{dk}_{cp}")
                    nc.sync.dma_start(out=x_t[:], in_=xT_ext[dk, cp])
                    row.append(x_t)
                xt.append(row)
            nc.vector.memset(ktz[:], 0.0)
            for dk in range(DK):
                nc.sync.dma_start(out=wv[:, dk, :], in_=wv_ext[dk])
            for t in range(2):
                nc.sync.dma_start(out=wo[:, t, :], in_=wo_ext[t])
            make_identity(nc, ident[:])

            # ---------------- QK projection + rope ----------------
            with ExitStack() as pctx:
                pj = pctx.enter_context(tc.tile_pool(name="proj", bufs=1))
                tmp = pctx.enter_context(tc.tile_pool(name="ptmp", bufs=4))
                pp = pctx.enter_context(tc.tile_pool(name="pjps", bufs=1, space="PSUM"))

                wqk = [pj.tile([P, 4 * P], BF16, tag=f"wqk{dk}", name=f"wqk{dk}")
                       for dk in range(DK)]
                cosf = pj.tile([P, L], F32, tag="cosf")
                sinf = pj.tile([P, L], F32, tag="sinf")
                qkr = pj.tile([P, 4, L], BF16, tag="qkr")  # qr1 qr2 kr1 kr2

                nc.sync.dma_start(out=cosf[:], in_=cos_ext[:])
                nc.sync.dma_start(out=sinf[:], in_=sin_ext[:])
                for dk in range(DK):
                    nc.sync.dma_start(out=wqk[dk][:], in_=wqk_ext[dk])

                # m: 0=qx1 1=qx2 2=kx1 3=kx2; dk-outer per chunk-pair
                for cp in range(2):
                    pq = {(c, m): pp.tile([P, NCHUNK], F32, tag=f"qk{c}{m}",
                                          name=f"pq_{cp}_{c}_{m}")
                          for c in range(2) for m in range(4)}
                    for dk in range(DK):
                        for c in range(2):
                            for m in range(4):
                                nc.tensor.matmul(
                                    pq[(c, m)][:],
                                    wqk[dk][:, m * P:(m + 1) * P],
                                    xt[dk][cp][:, c * NCHUNK:(c + 1) * NCHUNK],
                                    start=(dk == 0), stop=(dk == DK - 1))
                    for c in range(2):
                        xs = slice((2 * cp + c) * NCHUNK, (2 * cp + c + 1) * NCHUNK)
                        for base in (0, 2):
                            x1, x2 = pq[(c, base)], pq[(c, base + 1)]
                            t1 = tmp.tile([P, NCHUNK], F32, tag="t1")
                            t2 = tmp.tile([P, NCHUNK], F32, tag="t2")
                            nc.vector.tensor_mul(t1[:], x1[:], cosf[:, xs])
                            nc.vector.tensor_mul(t2[:], x2[:], sinf[:, xs])
                            nc.vector.tensor_sub(qkr[:, base, xs], t1[:], t2[:])
                            t3 = tmp.tile([P, NCHUNK], F32, tag="t1")
                            t4 = tmp.tile([P, NCHUNK], F32, tag="t2")
                            nc.vector.tensor_mul(t3[:], x1[:], sinf[:, xs])
                            nc.vector.tensor_mul(t4[:], x2[:], cosf[:, xs])
                            nc.vector.tensor_add(qkr[:, base + 1, xs], t3[:], t4[:])

                # head-contiguous Q^T (both heads per tile) and zero-padded K^T;
                # split per L-half so attention starts after the first chunk pair
                for lh in range(2):
                    ls = slice(lh * QH, (lh + 1) * QH)
                    for h in range(H):
                        t, pb = h // 2, 64 * (h % 2)
                        hs = slice(32 * h, 32 * h + 32)
                        nc.sync.dma_start(out=qt[pb:pb + 32, t, ls], in_=qkr[hs, 0, ls])
                        nc.sync.dma_start(out=qt[pb + 32:pb + 64, t, ls], in_=qkr[hs, 1, ls])
                        nc.sync.dma_start(out=ktz[pb:pb + 32, h, ls], in_=qkr[hs, 2, ls])
                        nc.sync.dma_start(out=ktz[pb + 32:pb + 64, h, ls], in_=qkr[hs, 3, ls])

            # ---------------- attention + finish, q-half-outer ----------------
            with ExitStack() as actx:
                ptp = actx.enter_context(tc.tile_pool(name="ptp", bufs=1))
                fin = actx.enter_context(tc.tile_pool(name="fin", bufs=1))
                aps = actx.enter_context(tc.tile_pool(name="aps", bufs=1, space="PSUM"))

                first_v = True
                for qh in range(2):
                    qhs = slice(qh * QH, (qh + 1) * QH)
                    o_nrm = fin.tile([P, QH // P, DL], BF16, tag="onrm", bufs=2)
                    for h in range(H):
                        t = h // 2
                        vs = slice(h * (HD + 1), (h + 1) * (HD + 1))
                        pts = []
                        for k2 in range(KT // 2):
                            pt2 = ptp.tile([P, 2, QH], BF16, tag="pt", bufs=12)
                            pts.append(pt2)
                            for ki in range(2):
                                k = 2 * k2 + ki
                                ks = slice(k * P, (k + 1) * P)
                                st = aps.tile([P, QH], F32, tag="st", bufs=2)
                                for qc in range(2):
                                    cs = slice(qc * NCHUNK, (qc + 1) * NCHUNK)
                                    qs = slice(qh * QH + qc * NCHUNK,
                                               qh * QH + (qc + 1) * NCHUNK)
                                    nc.tensor.matmul(
                                        st[:, cs], ktz[:, h, ks], qt[:, t, qs],
                                        start=True, stop=True)
                                nc.scalar.activation(
                                    pt2[:, ki, :], st[:],
                                    mybir.ActivationFunctionType.Exp)

                        if first_v:
                            # V projection, emitted here so it fills the PE
                            # while the first head's exps run
                            first_v = False
                            for k in range(KT):
                                pv = aps.tile([P, DL], F32, tag="misc", bufs=2)
                                for dk in range(DK):
                                    nc.tensor.matmul(
                                        pv[:],
                                        xt[dk][k // 8][:, (k % 8) * P:(k % 8 + 1) * P],
                                        wv[:, dk, :],
                                        start=(dk == 0), stop=(dk == DK - 1))
                                src3 = pv[:].rearrange("p (h d) -> p h d", h=H)
                                dst3 = v1[:, k, :].rearrange("p (h d) -> p h d", h=H)
                                nc.vector.tensor_copy(dst3[:, :, 0:HD], src3)
                                nc.vector.memset(dst3[:, :, HD:HD + 1], 1.0)

                        # PV: full-k accumulation chains, one per q-tile
                        for q in range(QH // P):
                            ob = aps.tile([P, HD + 1], F32, tag="ob", bufs=2)
                            for k in range(KT):
                                nc.tensor.matmul(
                                    ob[:], pts[k // 2][:, k % 2, q * P:(q + 1) * P],
                                    v1[:, k, vs],
                                    start=(k == 0), stop=(k == KT - 1))
                            rec = fin.tile([P, 1], F32, tag="rec", bufs=4)
                            nc.vector.reciprocal(rec[:], ob[:, HD:HD + 1])
                            nc.vector.tensor_scalar(
                                out=o_nrm[:, q, h * HD:(h + 1) * HD],
                                in0=ob[:, 0:HD],
                                scalar1=rec[:], scalar2=None,
                                op0=mybir.AluOpType.mult)

                    # transpose + out-proj + collective for this half
                    onT = fin.tile([P, 2, QH], BF16, tag="onT", bufs=2)
                    for q in range(QH // P):
                        for t in range(2):
                            ptr = aps.tile([P, P], BF16, tag="misc", bufs=2)
                            nc.tensor.transpose(
                                ptr[:], o_nrm[:, q, t * P:(t + 1) * P], ident[:])
                            nc.vector.tensor_copy(onT[:, t, q * P:(q + 1) * P], ptr[:])

                    for qcw in range(2):
                        ws = slice(qcw * NCHUNK, (qcw + 1) * NCHUNK)
                        for ot in range(DK):
                            po = aps.tile([P, NCHUNK], F32, tag="misc", bufs=2)
                            for t in range(2):
                                nc.tensor.matmul(
                                    po[:], wo[:, t, ot * P:(ot + 1) * P], onT[:, t, ws],
                                    start=(t == 0), stop=(t == 1))
                            so = fin.tile([P, NCHUNK], BF16, tag="so", bufs=4)
                            nc.vector.tensor_copy(so[:], po[:])
                            nc.sync.dma_start(
                                out=partials[qh][ot * P:(ot + 1) * P, ws], in_=so[:])
                        del po, so
                    nc.gpsimd.collective_compute(
                        "ReduceScatter", mybir.AluOpType.add,
                        replica_groups=GROUPS,
                        ins=[partials[qh][:]], outs=[scats[qh][:]])
                    nc.gpsimd.dma_start(out=out_ext[:, qhs], in_=scats[qh][:])

    nc.compile()
    return nc


def _prep_inputs(x, W_qkv, W_out):
    """Host-side sharding / layout prep -> per-core input maps."""
    Wq, Wk, Wv = W_qkv[0:D], W_qkv[D:2 * D], W_qkv[2 * D:3 * D]
    inv = 1.0 / (ROPE_BASE ** (np.arange(0, HD, 2, dtype=np.float64) / HD))
    pos = np.arange(L, dtype=np.float64)
    ang = pos[:, None] * inv[None, :]                     # [L, 32]
    cosF = np.tile(np.cos(ang).T, (H, 1)).astype(np.float32)  # [128, L]
    sinF = np.tile(np.sin(ang).T, (H, 1)).astype(np.float32)

    scale = float(HD) ** -0.5
    in_maps = []
    for c in range(8):
        b, g = c // 4, c % 4
        rows_x1 = np.array([64 * (4 * g + h) + 2 * f for h in range(H) for f in range(HF)])
        rows_x2 = rows_x1 + 1
        wqkT = np.concatenate([
            (scale * Wq[rows_x1]).T, (scale * Wq[rows_x2]).T,
            Wk[rows_x1].T, Wk[rows_x2].T], axis=1)        # [1024, 512]
        wvT = Wv[DL * g:DL * (g + 1)].T                   # [1024, 256]
        woT = W_out[:, DL * g:DL * (g + 1)].T             # [256, 1024]
        xTt = x[b].T.reshape(DK, P, 2, 2 * NCHUNK).transpose(0, 2, 1, 3)
        in_maps.append({
            "xT": np.ascontiguousarray(xTt).astype(ml_dtypes.bfloat16),
            "wqkT": np.ascontiguousarray(wqkT.reshape(DK, P, 4 * P)).astype(ml_dtypes.bfloat16),
            "wvT": np.ascontiguousarray(wvT.reshape(DK, P, DL)).astype(ml_dtypes.bfloat16),
            "woT": np.ascontiguousarray(woT.reshape(2, P, D)).astype(ml_dtypes.bfloat16),
            "cosF": cosF, "sinF": sinF,
        })
    return in_maps


def _run(in_maps, trace=False):
    global _CACHED_NC
    if _CACHED_NC is None:
        _CACHED_NC = _build_program()
    kw = dict(trace=True) if trace else {}
    return run_bass_kernel_spmd(_CACHED_NC, in_maps, list(range(8)), **kw)


def kernel(x, W_qkv, W_out, _trace=False):
    x = np.asarray(x, dtype=np.float32)
    W_qkv = np.asarray(W_qkv, dtype=np.float32)
    W_out = np.asarray(W_out, dtype=np.float32)
    res = _run(_prep_inputs(x, W_qkv, W_out), trace=_trace)
    out = np.empty((B, L, D), dtype=np.float32)
    for b in range(B):
        outT = np.concatenate([res.results[4 * b + j]["out"] for j in range(4)], axis=0)
        out[b] = outT.T
    if _trace:
        kernel.last_exec_time_ns = res.exec_time_ns
        kernel.last_trace = res.instructions_and_trace
    return out


# revision 8
# speedup vs baseline: 1.4457x; 1.0300x over previous
"""Multi-head attention (B=2, L=2048, D=1024, H=16, RoPE, softmax, out-proj)
on 8 Trainium2 NeuronCores.

Sharding: 2-way data parallel on batch x 4-way tensor parallel on heads.
Core c handles batch c//4 and heads 4*(c%4) .. 4*(c%4)+3. Each core:
  - projects its batch's activations with its head-slice of W_qkv
    (dk-outer accumulation so matmuls start as soon as x columns arrive)
  - applies RoPE, computes S^T = K_rope Q_rope^T per head in transposed
    layout (softmax normalizer and P.T@V then need no transposes of P);
    the K^T stationary operand is zero-padded to K=128 so the other head's
    lanes contribute exactly zero while the PE activity monitor sees a
    full-width matmul (K=64 matmuls leave the clock gated at half rate)
  - exp (no max subtraction: logits ~ N(0,1), |S| < ~6, exp is safe in fp32)
  - o~[q, 65] = P^T.T @ [V | 1] accumulated over all k in PSUM
    (col 64 = softmax denominator); normalized straight out of PSUM
  - q-half-outer loop: the first half's out-proj and ReduceScatter overlap
    the second half's attention
Host reassembles the full [2, 2048, 1024] output.

All matmuls bf16 with fp32 PSUM accumulation; softmax in fp32 (PSUM) with
bf16 P storage.
"""

import numpy as np
import ml_dtypes
from contextlib import ExitStack

import concourse.bass as bass
import concourse.tile as tile
from concourse import bacc, mybir
from concourse.bass_utils import run_bass_kernel_spmd
from concourse.masks import make_identity

BF16 = mybir.dt.bfloat16
F32 = mybir.dt.float32

B, L, D = 2, 2048, 1024
H_TOT, H = 16, 4          # total heads, heads per core
HD, HF = 64, 32           # head dim, rope freqs
DL = H * HD               # local head dims per core = 256
P = 128
KT = L // P               # 16 k-tiles
DK = D // P               # 8 contraction tiles over model dim
NCHUNK = 512
NQC = L // NCHUNK         # 4 sequence chunks (collective granularity)
QH = L // 2               # q half
ROPE_BASE = 10000.0
GROUPS = [[0, 1, 2, 3], [4, 5, 6, 7]]

_CACHED_NC = None


def _build_program():
    nc = bacc.Bacc("TRN2", target_bir_lowering=False, debug=False, num_devices=8)

    xT_ext = nc.dram_tensor("xT", [DK, 2, P, 2 * NCHUNK], BF16, kind="ExternalInput")
    wqk_ext = nc.dram_tensor("wqkT", [DK, P, 4 * P], BF16, kind="ExternalInput")
    wv_ext = nc.dram_tensor("wvT", [DK, P, DL], BF16, kind="ExternalInput")
    wo_ext = nc.dram_tensor("woT", [2, P, D], BF16, kind="ExternalInput")
    cos_ext = nc.dram_tensor("cosF", [P, L], F32, kind="ExternalInput")
    sin_ext = nc.dram_tensor("sinF", [P, L], F32, kind="ExternalInput")
    out_ext = nc.dram_tensor("out", [DL, L], F32, kind="ExternalOutput")

    partials = [nc.dram_tensor(f"partialT{c}", [D, QH], BF16) for c in range(2)]
    scats = [nc.dram_tensor(f"scatT{c}", [DL, QH], BF16) for c in range(2)]

    with tile.TileContext(nc) as tc:
        with ExitStack() as ctx:
            pers = ctx.enter_context(tc.tile_pool(name="pers", bufs=1))

            wv = pers.tile([P, DK, DL], BF16, tag="wv")
            wo = pers.tile([P, 2, D], BF16, tag="wo")
            qt = pers.tile([P, 2, L], BF16, tag="qt")      # head-contig Q^T
            ktz = pers.tile([P, H, L], BF16, tag="ktz")    # per-head zero-padded K^T
            v1 = pers.tile([P, KT, H * (HD + 1)], BF16, tag="v1")  # [V | 1]
            ident = pers.tile([P, P], BF16, tag="ident")

            xp = ctx.enter_context(tc.tile_pool(name="xp", bufs=1))
            xt = [[None, None] for _ in range(DK)]
            for cp in range(2):
                for dk in range(DK):
                    x_t = xp.tile([P, 2 * NCHUNK], BF16, tag=f"xt{dk}_{cp}",
                                  name=f"x_t{dk}_{cp}")
                    nc.sync.dma_start(out=x_t[:], in_=xT_ext[dk, cp])
                    xt[dk][cp] = x_t
                if cp == 0:
                    nc.vector.memset(ktz[:], 0.0)
            for dk in range(DK):
                nc.sync.dma_start(out=wv[:, dk, :], in_=wv_ext[dk])
            for t in range(2):
                nc.sync.dma_start(out=wo[:, t, :], in_=wo_ext[t])
            make_identity(nc, ident[:])

            # ---------------- QK projection + rope ----------------
            with ExitStack() as pctx:
                pj = pctx.enter_context(tc.tile_pool(name="proj", bufs=1))
                tmp = pctx.enter_context(tc.tile_pool(name="ptmp", bufs=4))
                pp = pctx.enter_context(tc.tile_pool(name="pjps", bufs=1, space="PSUM"))

                wqk = [pj.tile([P, 4 * P], BF16, tag=f"wqk{dk}", name=f"wqk{dk}")
                       for dk in range(DK)]
                cosf = pj.tile([P, L], F32, tag="cosf")
                sinf = pj.tile([P, L], F32, tag="sinf")
                qkr = pj.tile([P, 4, L], BF16, tag="qkr")  # qr1 qr2 kr1 kr2

                nc.sync.dma_start(out=cosf[:], in_=cos_ext[:])
                nc.sync.dma_start(out=sinf[:], in_=sin_ext[:])
                for dk in range(DK):
                    nc.sync.dma_start(out=wqk[dk][:], in_=wqk_ext[dk])

                # m: 0=qx1 1=qx2 2=kx1 3=kx2; dk-outer per chunk-pair
                for cp in range(2):
                    pq = {(c, m): pp.tile([P, NCHUNK], F32, tag=f"qk{c}{m}",
                                          name=f"pq_{cp}_{c}_{m}")
                          for c in range(2) for m in range(4)}
                    for dk in range(DK):
                        for c in range(2):
                            for m in range(4):
                                nc.tensor.matmul(
                                    pq[(c, m)][:],
                                    wqk[dk][:, m * P:(m + 1) * P],
                                    xt[dk][cp][:, c * NCHUNK:(c + 1) * NCHUNK],
                                    start=(dk == 0), stop=(dk == DK - 1))
                    for c in range(2):
                        xs = slice((2 * cp + c) * NCHUNK, (2 * cp + c + 1) * NCHUNK)
                        for base in (0, 2):
                            x1, x2 = pq[(c, base)], pq[(c, base + 1)]
                            t1 = tmp.tile([P, NCHUNK], F32, tag="t1")
                            t2 = tmp.tile([P, NCHUNK], F32, tag="t2")
                            nc.vector.tensor_mul(t1[:], x1[:], cosf[:, xs])
                            nc.vector.tensor_mul(t2[:], x2[:], sinf[:, xs])
                            nc.vector.tensor_sub(qkr[:, base, xs], t1[:], t2[:])
                            t3 = tmp.tile([P, NCHUNK], F32, tag="t1")
                            t4 = tmp.tile([P, NCHUNK], F32, tag="t2")
                            nc.vector.tensor_mul(t3[:], x1[:], sinf[:, xs])
                            nc.vector.tensor_mul(t4[:], x2[:], cosf[:, xs])
                            nc.vector.tensor_add(qkr[:, base + 1, xs], t3[:], t4[:])

                # head-contiguous Q^T (both heads per tile) and zero-padded K^T;
                # split per L-half so attention starts after the first chunk pair
                for lh in range(2):
                    ls = slice(lh * QH, (lh + 1) * QH)
                    for h in range(H):
                        t, pb = h // 2, 64 * (h % 2)
                        hs = slice(32 * h, 32 * h + 32)
                        nc.sync.dma_start(out=qt[pb:pb + 32, t, ls], in_=qkr[hs, 0, ls])
                        nc.sync.dma_start(out=qt[pb + 32:pb + 64, t, ls], in_=qkr[hs, 1, ls])
                        nc.sync.dma_start(out=ktz[pb:pb + 32, h, ls], in_=qkr[hs, 2, ls])
                        nc.sync.dma_start(out=ktz[pb + 32:pb + 64, h, ls], in_=qkr[hs, 3, ls])

            # ---------------- attention + finish, q-half-outer ----------------
            with ExitStack() as actx:
                ptp = actx.enter_context(tc.tile_pool(name="ptp", bufs=1))
                fin = actx.enter_context(tc.tile_pool(name="fin", bufs=1))
                aps = actx.enter_context(tc.tile_pool(name="aps", bufs=1, space="PSUM"))

                first_v = True
                for qh in range(2):
                    qhs = slice(qh * QH, (qh + 1) * QH)
                    o_nrm = fin.tile([P, QH // P, DL], BF16, tag="onrm", bufs=2)
                    for h in range(H):
                        t = h // 2
                        vs = slice(h * (HD + 1), (h + 1) * (HD + 1))
                        pts = []
                        for k2 in range(KT // 2):
                            pt2 = ptp.tile([P, 2, QH], BF16, tag="pt", bufs=12)
                            pts.append(pt2)
                            for ki in range(2):
                                k = 2 * k2 + ki
                                ks = slice(k * P, (k + 1) * P)
                                st = aps.tile([P, QH], F32, tag="st", bufs=2)
                                for qc in range(2):
                                    cs = slice(qc * NCHUNK, (qc + 1) * NCHUNK)
                                    qs = slice(qh * QH + qc * NCHUNK,
                                               qh * QH + (qc + 1) * NCHUNK)
                                    nc.tensor.matmul(
                                        st[:, cs], ktz[:, h, ks], qt[:, t, qs],
                                        start=True, stop=True)
                                nc.scalar.activation(
                                    pt2[:, ki, :], st[:],
                                    mybir.ActivationFunctionType.Exp)

                        if first_v:
                            # V projection, emitted here so it fills the PE
                            # while the first head's exps run
                            first_v = False
                            for k in range(KT):
                                pv = aps.tile([P, DL], F32, tag="misc", bufs=2)
                                for dk in range(DK):
                                    nc.tensor.matmul(
                                        pv[:],
                                        xt[dk][k // 8][:, (k % 8) * P:(k % 8 + 1) * P],
                                        wv[:, dk, :],
                                        start=(dk == 0), stop=(dk == DK - 1))
                                src3 = pv[:].rearrange("p (h d) -> p h d", h=H)
                                dst3 = v1[:, k, :].rearrange("p (h d) -> p h d", h=H)
                                nc.vector.tensor_copy(dst3[:, :, 0:HD], src3)
                                nc.vector.memset(dst3[:, :, HD:HD + 1], 1.0)

                        # PV: full-k accumulation chains, one per q-tile
                        for q in range(QH // P):
                            ob = aps.tile([P, HD + 1], F32, tag="ob", bufs=2)
                            for k in range(KT):
                                nc.tensor.matmul(
                                    ob[:], pts[k // 2][:, k % 2, q * P:(q + 1) * P],
                                    v1[:, k, vs],
                                    start=(k == 0), stop=(k == KT - 1))
                            rec = fin.tile([P, 1], F32, tag="rec", bufs=4)
                            nc.vector.reciprocal(rec[:], ob[:, HD:HD + 1])
                            nc.vector.tensor_scalar(
                                out=o_nrm[:, q, h * HD:(h + 1) * HD],
                                in0=ob[:, 0:HD],
                                scalar1=rec[:], scalar2=None,
                                op0=mybir.AluOpType.mult)

                    # transpose + out-proj + collective for this half
                    onT = fin.tile([P, 2, QH], BF16, tag="onT", bufs=2)
                    for q in range(QH // P):
                        for t in range(2):
                            ptr = aps.tile([P, P], BF16, tag="misc", bufs=2)
                            nc.tensor.transpose(
                                ptr[:], o_nrm[:, q, t * P:(t + 1) * P], ident[:])
                            nc.vector.tensor_copy(onT[:, t, q * P:(q + 1) * P], ptr[:])

                    for qcw in range(2):
                        ws = slice(qcw * NCHUNK, (qcw + 1) * NCHUNK)
                        for ot in range(DK):
                            po = aps.tile([P, NCHUNK], F32, tag="misc", bufs=2)
                            for t in range(2):
                                nc.tensor.matmul(
                                    po[:], wo[:, t, ot * P:(ot + 1) * P], onT[:, t, ws],
                                    start=(t == 0), stop=(t == 1))
                            so = fin.tile([P, NCHUNK], BF16, tag="so", bufs=4)
                            nc.vector.tensor_copy(so[:], po[:])
                            nc.sync.dma_start(
                                out=partials[qh][ot * P:(ot + 1) * P, ws], in_=so[:])
                        del po, so
                    nc.gpsimd.collective_compute(
                        "ReduceScatter", mybir.AluOpType.add,
                        replica_groups=GROUPS,
                        ins=[partials[qh][:]], outs=[scats[qh][:]])
                    nc.gpsimd.dma_start(out=out_ext[:, qhs], in_=scats[qh][:])

    nc.compile()
    return nc


def _prep_inputs(x, W_qkv, W_out):
    """Host-side sharding / layout prep -> per-core input maps."""
    Wq, Wk, Wv = W_qkv[0:D], W_qkv[D:2 * D], W_qkv[2 * D:3 * D]
    inv = 1.0 / (ROPE_BASE ** (np.arange(0, HD, 2, dtype=np.float64) / HD))
    pos = np.arange(L, dtype=np.float64)
    ang = pos[:, None] * inv[None, :]                     # [L, 32]
    cosF = np.tile(np.cos(ang).T, (H, 1)).astype(np.float32)  # [128, L]
    sinF = np.tile(np.sin(ang).T, (H, 1)).astype(np.float32)

    scale = float(HD) ** -0.5
    in_maps = []
    for c in range(8):
        b, g = c // 4, c % 4
        rows_x1 = np.array([64 * (4 * g + h) + 2 * f for h in range(H) for f in range(HF)])
        rows_x2 = rows_x1 + 1
        wqkT = np.concatenate([
            (scale * Wq[rows_x1]).T, (scale * Wq[rows_x2]).T,
            Wk[rows_x1].T, Wk[rows_x2].T], axis=1)        # [1024, 512]
        wvT = Wv[DL * g:DL * (g + 1)].T                   # [1024, 256]
        woT = W_out[:, DL * g:DL * (g + 1)].T             # [256, 1024]
        xTt = x[b].T.reshape(DK, P, 2, 2 * NCHUNK).transpose(0, 2, 1, 3)
        in_maps.append({
            "xT": np.ascontiguousarray(xTt).astype(ml_dtypes.bfloat16),
            "wqkT": np.ascontiguousarray(wqkT.reshape(DK, P, 4 * P)).astype(ml_dtypes.bfloat16),
            "wvT": np.ascontiguousarray(wvT.reshape(DK, P, DL)).astype(ml_dtypes.bfloat16),
            "woT": np.ascontiguousarray(woT.reshape(2, P, D)).astype(ml_dtypes.bfloat16),
            "cosF": cosF, "sinF": sinF,
        })
    return in_maps


def _run(in_maps, trace=False):
    global _CACHED_NC
    if _CACHED_NC is None:
        _CACHED_NC = _build_program()
    kw = dict(trace=True) if trace else {}
    return run_bass_kernel_spmd(_CACHED_NC, in_maps, list(range(8)), **kw)


def kernel(x, W_qkv, W_out, _trace=False):
    x = np.asarray(x, dtype=np.float32)
    W_qkv = np.asarray(W_qkv, dtype=np.float32)
    W_out = np.asarray(W_out, dtype=np.float32)
    res = _run(_prep_inputs(x, W_qkv, W_out), trace=_trace)
    out = np.empty((B, L, D), dtype=np.float32)
    for b in range(B):
        outT = np.concatenate([res.results[4 * b + j]["out"] for j in range(4)], axis=0)
        out[b] = outT.T
    if _trace:
        kernel.last_exec_time_ns = res.exec_time_ns
        kernel.last_trace = res.instructions_and_trace
    return out
